# revision 44
# baseline (speedup 1.0000x reference)
"""Trainium2 Bass kernel for fused multi-head causal attention (GPT-2 style).

Full-input contract: kernel(**inputs) takes the complete tensors and returns
the complete output. Internally: data-parallel over the batch dim (B=8) across
8 NeuronCores; each core runs the whole attention block for one batch element.

Per-core dataflow (S=512, D=1024, H=16, dh=64). All matmul operands are bf16
(PSUM accumulation stays fp32); rel-err tolerance is 2e-2 so bf16 rounding
(~0.5%) is safe and doubles effective PE/DMA throughput vs fp32:

  x^T [128, d, S]   host-packed bf16, resident in SBUF (2 DMA queues)
  V:   psum[s,n]   = x^T[:,s].T @ W1v        -> [V|1]/[1|V] head blocks in SBUF
  QK:  psum[n,s]   = W1qk[:,n].T @ x^T       -> Q^T,K^T   (bias on ACT engine)
  S^T: psum[sk,sq] = K_h^T[:,sk].T @ Q_h^T   (scores transposed, causal-trimmed)
  P^T  = exp(S^T/8 + pad_bias) on ACT; tril multiply on DVE zeroes the future
  A^T: psum = [V_h|1].T @ P^T                (64 attn rows + 64 denominator rows)
  A^T_norm = psum_attn * recip_approx(psum_denom)  -> A^T bf16 tiles [n, s]
  out: psum[s,e]   = A^T[:,s].T @ W2         + b2 (fp32 out)

Schedule is ordered by DMA arrival: QK pass 0 (needs x^T + 0.5MB of weights)
runs first, then the V projection c-half 0; head pairs are software-pipelined
with the next QK pass (or V c-half 1) emitted between a pair's score matmuls
and its A^T matmuls so the PE never waits on the ACT-engine exps.
"""

import sys

if "/opt/trn_rl_repo" not in sys.path:
    sys.path.insert(0, "/opt/trn_rl_repo")

import numpy as np

import concourse.bass as bass
import concourse.mybir as mybir
import concourse.tile as tile
from concourse import bacc
from concourse.bass_utils import run_bass_kernel_spmd
from concourse.masks import make_upper_triangular

F32 = mybir.dt.float32
BF16 = mybir.dt.bfloat16
NPBF16 = mybir.dt.np(BF16)

B, S, D = 8, 512, 1024
H = 16
DH = D // H          # 64
NT_S = S // 128      # 4 s-tiles
ND = D // 128        # 8 d-tiles
N_CORES = 8
SCALE = 1.0 / 8.0    # 1/sqrt(head_dim)

_CACHED = {}


def _dram_ap(t, offset, dims):
    """Raw strided DRAM access pattern ([step, count] pairs, elements)."""
    return bass.AP(tensor=t[...].tensor, offset=offset, ap=dims)


def _build_nc(dbg=False):
    nc = bacc.Bacc("TRN2", target_bir_lowering=False, debug=False)

    Exp = mybir.ActivationFunctionType.Exp
    Ident = mybir.ActivationFunctionType.Identity
    ADD = mybir.AluOpType.add
    MUL = mybir.AluOpType.mult

    dbg_d = {}
    if dbg:
        dbg_d["dva"] = nc.dram_tensor("dva", [128, NT_S, D], BF16,
                                      kind="ExternalOutput")
        dbg_d["dat"] = nc.dram_tensor("dat", [128, ND, S], BF16,
                                      kind="ExternalOutput")
        dbg_d["dqt"] = nc.dram_tensor("dqt", [128, S], BF16, kind="ExternalOutput")
        dbg_d["dkt"] = nc.dram_tensor("dkt", [128, S], BF16, kind="ExternalOutput")
        dbg_d["dpt"] = nc.dram_tensor("dpt", [NT_S, 128, 2, 512], BF16,
                                      kind="ExternalOutput")

    # host-packed bf16 inputs (all [128 partitions, ...] contiguous)
    xt_lo = nc.dram_tensor("xt_lo", [128, 4, S], BF16, kind="ExternalInput")
    xt_hi = nc.dram_tensor("xt_hi", [128, 4, S], BF16, kind="ExternalInput")
    w1v0_d = nc.dram_tensor("w1v0", [128, ND, 512], BF16, kind="ExternalInput")
    w1v1_d = nc.dram_tensor("w1v1", [128, ND, 512], BF16, kind="ExternalInput")
    wqk_pk = nc.dram_tensor("wqk_pk", [ND, 128, ND, 2, 128], BF16,
                            kind="ExternalInput")
    w2_d = nc.dram_tensor("w2pk", [128, ND, D], BF16, kind="ExternalInput")
    # host-packed [128, n] consts — a strided gather here costs 128 DMA
    # descriptors and clogs the DMA engines at startup
    msk_d = nc.dram_tensor("mskpk", [128, NT_S], F32, kind="ExternalInput")
    bqk_d = nc.dram_tensor("bqkpk", [128, 2 * ND], F32, kind="ExternalInput")
    bv_d = nc.dram_tensor("bvpk", [128, D], BF16, kind="ExternalInput")
    bo_d = nc.dram_tensor("bopk", [128, D], F32, kind="ExternalInput")
    out = nc.dram_tensor("out", [S, D], F32, kind="ExternalOutput")

    with tile.TileContext(nc) as tc:
        with (
            tc.tile_pool(name="const", bufs=1) as const_p,
            tc.tile_pool(name="xt", bufs=1) as xt_p,
            tc.tile_pool(name="w1v", bufs=2) as w1v_p,
            tc.tile_pool(name="wqk", bufs=8) as wqk_p,
            tc.tile_pool(name="qkt", bufs=6) as qkt_p,
            tc.tile_pool(name="vsb", bufs=1) as vsb_p,
            tc.tile_pool(name="pt", bufs=9) as pt_p,
            tc.tile_pool(name="rc", bufs=2) as rc_p,
            tc.tile_pool(name="at", bufs=1) as at_p,
            tc.tile_pool(name="w2sb", bufs=1) as w2sb_p,
            tc.tile_pool(name="outsb", bufs=3) as out_p,
            tc.tile_pool(name="mmps", bufs=2, space="PSUM") as mmps_p,
            tc.tile_pool(name="scps", bufs=2, space="PSUM") as scps_p,
            tc.tile_pool(name="avps", bufs=1, space="PSUM") as avps_p,
            tc.tile_pool(name="dnps", bufs=1, space="PSUM") as dnps_p,
        ):
            # ---------- prefetch DMAs; per-queue FIFO order == need order ----
            xt_sb = xt_p.tile([128, ND, S], BF16, tag="xt")
            wqks = [wqk_p.tile([128, ND, 2, 128], BF16, tag="wqk",
                               name=f"wqk{i}") for i in range(ND)]
            w1v0 = w1v_p.tile([128, ND, 512], BF16, tag="w1v", name="w1v0")
            w1v1 = w1v_p.tile([128, ND, 512], BF16, tag="w1v", name="w1v1")
            w2sb = w2sb_p.tile([128, ND, D], BF16, tag="w2sb")
            va_sb = vsb_p.tile([128, NT_S, D], BF16, tag="va")
            at_sb = at_p.tile([128, ND, S], BF16, tag="at")

            # ACT-table warm-up: first Exp otherwise pays ~1.3us table load
            warm = const_p.tile([128, 1], F32)
            nc.vector.memset(warm[:], 0.0)
            nc.scalar.activation(warm[:], warm[:],
                                 mybir.ActivationFunctionType.Exp)

            # 3 DMA queues: sync(SP), scalar(ACT), gpsimd. FIFO per queue,
            # ordered by first use: QK0/QK1 need xt+wqk, then V needs w1v0.
            bqk = const_p.tile([128, 2 * ND], F32)
            msk_sb = const_p.tile([128, NT_S], F32)
            bv = const_p.tile([128, D], BF16)
            bo = const_p.tile([128, D], F32)
            nc.gpsimd.dma_start(bqk[:], bqk_d[...])
            nc.gpsimd.dma_start(msk_sb[:], msk_d[...])
            nc.sync.dma_start(xt_sb[:, 0:4, :], xt_lo[...])
            nc.scalar.dma_start(wqks[0][:], wqk_pk[0])
            nc.gpsimd.dma_start(xt_sb[:, 4:8, :], xt_hi[...])
            nc.sync.dma_start(wqks[1][:], wqk_pk[1])
            nc.sync.dma_start(wqks[2][:], wqk_pk[2])
            nc.sync.dma_start(w1v0[:, 0:4, :], w1v0_d[:, 0:4, :])
            nc.scalar.dma_start(w1v0[:, 4:8, :], w1v0_d[:, 4:8, :])
            nc.gpsimd.dma_start(bv[:], bv_d[...])

            for i in (4, 6):
                nc.sync.dma_start(wqks[i][:], wqk_pk[i])
            for i in (3, 5, 7):
                nc.gpsimd.dma_start(wqks[i][:], wqk_pk[i])
            nc.scalar.dma_start(w1v1[:], w1v1_d[...])
            nc.scalar.dma_start(w2sb[:], w2_d[...])
            nc.scalar.dma_start(bo[:], bo_d[...])

            # ones column block for the denominator matmuls
            ones64 = const_p.tile([128, 64], BF16)
            nc.gpsimd.memset(ones64[:], 1.0)
            # keep[k, q] = 1 if q >= k else 0, replicated for both heads
            tril2 = const_p.tile([128, 2, 128], BF16)
            make_upper_triangular(nc, tril2[:, 0, :], val=1.0, diag=True)
            make_upper_triangular(nc, tril2[:, 1, :], val=1.0, diag=True)
            pad_bias = const_p.tile([128, NT_S], F32)
            nc.vector.tensor_scalar(
                out=pad_bias[:], in0=msk_sb[:], scalar1=1.0, scalar2=1e9,
                op0=mybir.AluOpType.subtract, op1=mybir.AluOpType.mult,
            )

            # ---------------- emit helpers ----------------
            def emit_qk_pass(i):
                psq = mmps_p.tile([128, 512], F32, tag="mmps")
                psk = mmps_p.tile([128, 512], F32, tag="mmps")
                for d in range(ND):
                    nc.tensor.matmul(psq[:], wqks[i][:, d, 0, :], xt_sb[:, d, :],
                                     start=(d == 0), stop=(d == ND - 1))
                    nc.tensor.matmul(psk[:], wqks[i][:, d, 1, :], xt_sb[:, d, :],
                                     start=(d == 0), stop=(d == ND - 1))
                qt = qkt_p.tile([128, S], BF16, tag="qkt")
                kt = qkt_p.tile([128, S], BF16, tag="qkt")
                nc.scalar.activation(qt[:], psq[:], Ident,
                                     bias=bqk[:, i : i + 1], scale=1.0)
                nc.scalar.activation(kt[:], psk[:], Ident,
                                     bias=bqk[:, ND + i : ND + i + 1], scale=1.0)
                return qt, kt

            def emit_v(c, w1v_c):
                # V projection c-half: heads 8c..8c+7 -> va cols c*512..+512
                for t in range(NT_S):
                    ps = mmps_p.tile([128, 512], F32, tag="mmps")
                    for d in range(ND):
                        nc.tensor.matmul(ps[:], xt_sb[:, d, t * 128 : (t + 1) * 128],
                                         w1v_c[:, d, :],
                                         start=(d == 0), stop=(d == ND - 1))
                    # (gpsimd cannot read PSUM -> all psum-draining ops on DVE)
                    nc.vector.tensor_tensor(
                        out=va_sb[:, t, c * 512 : (c + 1) * 512], in0=ps[:],
                        in1=bv[:, c * 512 : (c + 1) * 512], op=ADD)

            def emit_scores(i, qt, kt):
                pts = []
                for sk in range(NT_S):
                    w = S - sk * 128
                    sc = scps_p.tile([128, 2, 512], F32, tag="scps")
                    nc.tensor.matmul(sc[:, 0, 0:w], kt[0:64, sk * 128 : (sk + 1) * 128],
                                     qt[0:64, sk * 128 : S], start=True, stop=True)
                    nc.tensor.matmul(sc[:, 1, 0:w], kt[64:128, sk * 128 : (sk + 1) * 128],
                                     qt[64:128, sk * 128 : S], start=True, stop=True)
                    pt = pt_p.tile([128, 2, 512], BF16, tag="pt")
                    nc.scalar.activation(pt[:, :, 0:w], sc[:, :, 0:w], Exp,
                                         bias=pad_bias[:, sk : sk + 1], scale=SCALE)
                    # zero strictly-future entries of the diagonal block
                    nc.vector.tensor_tensor(out=pt[:, :, 0:128], in0=pt[:, :, 0:128],
                                            in1=tril2[:], op=MUL)
                    pts.append(pt)
                return pts

            def emit_av(i, pts):
                h_e, h_o = 2 * i, 2 * i + 1
                # packed denominators: den_e on partitions 0:64, den_o on
                # 64:128 of ONE tile -> a single unshifted reciprocal
                dn = dnps_p.tile([128, 512], F32, tag="dnps")
                av2 = avps_p.tile([128, 512], F32, tag="avps")
                for sk in range(NT_S):
                    w = S - sk * 128
                    nc.tensor.matmul(dn[0:64, sk * 128 : S], ones64[:],
                                     pts[sk][:, 0, 0:w],
                                     start=(sk == 0), stop=(sk == NT_S - 1))
                    nc.tensor.matmul(dn[64:128, sk * 128 : S], ones64[:],
                                     pts[sk][:, 1, 0:w],
                                     start=(sk == 0), stop=(sk == NT_S - 1))
                # chunked recip+normalize for the last pairs: c_proj reads
                # at[:, i, :] in 128-col slices, so let them unblock
                # progressively (these norms sit in the c_proj critical path)
                nch = 4 if i >= ND - 2 else 1
                cw = 512 // nch
                rc = rc_p.tile([128, 512], F32, tag="rc")
                for ch in range(nch):
                    nc.vector.reciprocal(out=rc[:, ch * cw : (ch + 1) * cw],
                                         in_=dn[:, ch * cw : (ch + 1) * cw])
                for sk in range(NT_S):
                    w = S - sk * 128
                    nc.tensor.matmul(av2[0:64, sk * 128 : S],
                                     va_sb[:, sk, h_e * 64 : h_e * 64 + 64],
                                     pts[sk][:, 0, 0:w],
                                     start=(sk == 0), stop=(sk == NT_S - 1))
                    nc.tensor.matmul(av2[64:128, sk * 128 : S],
                                     va_sb[:, sk, h_o * 64 : h_o * 64 + 64],
                                     pts[sk][:, 1, 0:w],
                                     start=(sk == 0), stop=(sk == NT_S - 1))
                for ch in range(nch):
                    sl = slice(ch * cw, (ch + 1) * cw)
                    nc.vector.tensor_tensor(out=at_sb[:, i, sl],
                                            in0=av2[:, sl], in1=rc[:, sl],
                                            op=MUL)

            # ---------------- schedule ----------------
            # Front-load QK passes 0-2 + scores(0) so the PE has work while
            # the V weights stream in; V_c0 right before the pair loop.
            qt0, kt0 = emit_qk_pass(0)
            qts = {1: emit_qk_pass(1)}
            if dbg:
                nc.sync.dma_start(dbg_d["dqt"][...], qt0[:])
                nc.sync.dma_start(dbg_d["dkt"][...], kt0[:])
            pts0 = emit_scores(0, qt0, kt0)
            if dbg:
                for sk in range(NT_S):
                    nc.sync.dma_start(dbg_d["dpt"][sk], pts0[sk][:])
            qts[2] = emit_qk_pass(2)
            emit_v(0, w1v0)
            next_qk = 3
            for i in range(ND):
                pts = pts0 if i == 0 else emit_scores(i, *qts.pop(i))
                # independent PE work between scores and A^T hides the exps;
                # V_c1 at i=3: late enough for its DMA, before pair 4 needs it
                if i == 3:
                    emit_v(1, w1v1)
                elif i >= 1 and next_qk < ND:
                    qts[next_qk] = emit_qk_pass(next_qk)
                    next_qk += 1
                elif i == ND - 1:
                    # hide the last pair's exps + reciprocal: run the first
                    # four c_proj chains' d=0..6 steps now, d=7 after norm(7).
                    # t=1 chains borrow the scores PSUM pool (idle after the
                    # final exps drain).
                    cpre = []
                    for c in range(2):
                        ps = mmps_p.tile([128, 512], F32, tag="mmps")
                        for d in range(ND - 1):
                            nc.tensor.matmul(ps[:], at_sb[:, d, 0:128],
                                             w2sb[:, d, c * 512 : (c + 1) * 512],
                                             start=(d == 0), stop=False)
                        cpre.append(ps)
                    cpre2 = scps_p.tile([128, 2, 512], F32, tag="scps")
                    for c in range(2):
                        for d in range(ND - 1):
                            nc.tensor.matmul(cpre2[:, c, :],
                                             at_sb[:, d, 128:256],
                                             w2sb[:, d, c * 512 : (c + 1) * 512],
                                             start=(d == 0), stop=False)
                emit_av(i, pts)

            if dbg:
                nc.sync.dma_start(dbg_d["dva"][...], va_sb[:])
                nc.sync.dma_start(dbg_d["dat"][...], at_sb[:])

            # ---------------- c_proj ----------------
            for t in range(NT_S):
                for c in range(2):
                    if t == 0:
                        ps = cpre[c]
                        nc.tensor.matmul(ps[:], at_sb[:, ND - 1, 0:128],
                                         w2sb[:, ND - 1, c * 512 : (c + 1) * 512],
                                         start=False, stop=True)
                    elif t == 1:
                        ps = cpre2[:, c, :]
                        nc.tensor.matmul(ps, at_sb[:, ND - 1, 128:256],
                                         w2sb[:, ND - 1, c * 512 : (c + 1) * 512],
                                         start=False, stop=True)
                    else:
                        ps = mmps_p.tile([128, 512], F32, tag="mmps")
                        for d in range(ND):
                            nc.tensor.matmul(ps[:],
                                             at_sb[:, d, t * 128 : (t + 1) * 128],
                                             w2sb[:, d, c * 512 : (c + 1) * 512],
                                             start=(d == 0), stop=(d == ND - 1))
                    ob = out_p.tile([128, 512], F32, tag="outsb")
                    # final tile: chunk bias+store so the last DMA is small
                    # and the drain tail shrinks
                    nst = 2 if t == NT_S - 1 else 1
                    sw = 512 // nst
                    for st in range(nst):
                        sl = slice(st * sw, (st + 1) * sw)
                        nc.vector.tensor_tensor(
                            out=ob[:, sl], in0=ps[:, sl],
                            in1=bo[:, c * 512 + st * sw : c * 512 + (st + 1) * sw],
                            op=ADD)
                        # sync engine is idle at the tail; ACT would delay
                        # these issues behind the queued exps
                        nc.sync.dma_start(
                            out[t * 128 : (t + 1) * 128,
                                c * 512 + st * sw : c * 512 + (st + 1) * sw],
                            ob[:, sl])

    nc.compile()
    return nc


def _get_nc(dbg=False):
    key = ("nc", dbg)
    if key not in _CACHED:
        _CACHED[key] = _build_nc(dbg)
    return _CACHED[key]


def _pack_weights(w1, w2):
    """Host-side bf16 packing into [128, ...] contiguous DMA blocks."""
    w1r = w1.reshape(ND, 128, 3 * D)
    qs = w1r[:, :, :D].reshape(ND, 128, ND, 128)       # [d, p, i, c]
    ks = w1r[:, :, D : 2 * D].reshape(ND, 128, ND, 128)
    pk = np.stack([qs, ks], axis=3)                    # [d, p, i, {q,k}, c]
    wqk_pk = pk.transpose(2, 1, 0, 3, 4).astype(NPBF16)  # [i, p, d, {q,k}, c]
    w1v = w1r[:, :, 2 * D :]                           # [d, p, 1024]
    w1v0 = w1v[:, :, :512].transpose(1, 0, 2).astype(NPBF16)   # [p, d, 512]
    w1v1 = w1v[:, :, 512:].transpose(1, 0, 2).astype(NPBF16)
    w2pk = w2.reshape(ND, 128, D).transpose(1, 0, 2).astype(NPBF16)  # [p, d, e]
    return (np.ascontiguousarray(wqk_pk), np.ascontiguousarray(w1v0),
            np.ascontiguousarray(w1v1), np.ascontiguousarray(w2pk))


def _make_in_maps(x, mask, w1, b1, w2, b2):
    wqk_pk, w1v0, w1v1, w2pk = _pack_weights(w1, w2)
    bqk_pk = np.ascontiguousarray(b1[: 2 * D].reshape(2 * ND, 128).T)
    bv_pk = np.ascontiguousarray(
        np.broadcast_to(b1[2 * D :], (128, D)).astype(NPBF16))
    bo_pk = np.ascontiguousarray(np.broadcast_to(b2, (128, D)))
    in_maps = []
    for b in range(N_CORES):
        xtp = x[b].T.reshape(ND, 128, S).transpose(1, 0, 2)  # [p, d, s]
        in_maps.append({
            "xt_lo": np.ascontiguousarray(xtp[:, 0:4].astype(NPBF16)),
            "xt_hi": np.ascontiguousarray(xtp[:, 4:8].astype(NPBF16)),
            "w1v0": w1v0, "w1v1": w1v1, "wqk_pk": wqk_pk, "w2pk": w2pk,
            "mskpk": np.ascontiguousarray(mask[b].reshape(NT_S, 128).T),
            "bqkpk": bqk_pk, "bvpk": bv_pk, "bopk": bo_pk,
        })
    return in_maps


def _trace_setup(inputs):
    """Build (nc, in_maps) exactly as kernel() would — for test.py tracing."""
    x = np.asarray(inputs["x"], dtype=np.float32)
    mask = np.asarray(inputs["mask"], dtype=np.float32)
    w1 = np.ascontiguousarray(np.asarray(inputs["c_attn_w"], dtype=np.float32))
    b1 = np.ascontiguousarray(np.asarray(inputs["c_attn_b"], dtype=np.float32))
    w2 = np.ascontiguousarray(np.asarray(inputs["c_proj_w"], dtype=np.float32))
    b2 = np.ascontiguousarray(np.asarray(inputs["c_proj_b"], dtype=np.float32))
    return _get_nc(), _make_in_maps(x, mask, w1, b1, w2, b2)


def kernel(x, mask, c_attn_w, c_attn_b, c_proj_w, c_proj_b):
    x = np.asarray(x, dtype=np.float32)
    mask = np.asarray(mask, dtype=np.float32)
    w1 = np.ascontiguousarray(np.asarray(c_attn_w, dtype=np.float32))
    b1 = np.ascontiguousarray(np.asarray(c_attn_b, dtype=np.float32))
    w2 = np.ascontiguousarray(np.asarray(c_proj_w, dtype=np.float32))
    b2 = np.ascontiguousarray(np.asarray(c_proj_b, dtype=np.float32))

    nc = _get_nc()
    in_maps = _make_in_maps(x, mask, w1, b1, w2, b2)
    res = run_bass_kernel_spmd(nc, in_maps, list(range(N_CORES)))
    return np.stack([res.results[b]["out"] for b in range(N_CORES)], axis=0)
